# revision 8
# baseline (speedup 1.0000x reference)
import numpy as np
import concourse.bacc as bacc
import concourse.mybir as mybir
from concourse import tile
from concourse.bass_utils import run_bass_kernel_spmd

dt = mybir.dt
AF = mybir.ActivationFunctionType
OP = mybir.AluOpType

EPS = 1e-5
B, C, H, W = 2, 512, 48, 48
N = H * W
NC = 19
CORES = list(range(8))
PROFILE = False

GROUPS1 = [(0, 5), (5, 5), (10, 5), (15, 5), (20, 4)]   # conv output row groups (local)
PASSES1 = [(0, 1), (2, 3), (4,)]
GROUPS2 = [(1, 5), (6, 5), (11, 5), (16, 5), (21, 4)]   # conv2 out rows in local up coords


def _build_l1():
    nc = bacc.Bacc("TRN2", target_bir_lowering=False, num_devices=8)
    x2s = nc.dram_tensor("x2s", [8, 128, 26, 98], dt.float32r, kind="ExternalInput").ap()
    w1 = nc.dram_tensor("w1", [72, 128, 512], dt.float32r, kind="ExternalInput").ap()
    w2 = nc.dram_tensor("w2", [4, 128, NC], dt.float32r, kind="ExternalInput").ap()
    f3 = nc.dram_tensor("f3", [128, 4], dt.float32, kind="ExternalInput").ap()
    b3 = nc.dram_tensor("b3", [128, 4], dt.float32, kind="ExternalInput").ap()
    b2 = nc.dram_tensor("b2", [NC, 1], dt.float32, kind="ExternalInput").ap()
    aux2_sh = nc.dram_tensor("aux2_sh", [NC, 24, 96], dt.float32, kind="ExternalOutput").ap()
    with tile.TileContext(nc) as tc:
        with (
            tc.tile_pool(name="big", bufs=1) as big,
            tc.tile_pool(name="wp", bufs=4) as wp,
            tc.tile_pool(name="cp", bufs=8, space="PSUM") as cp,
            tc.tile_pool(name="sm", bufs=1) as sm,
        ):
            x2t = big.tile([128, 8, 26, 98], dt.float32r)
            for kt in range(8):
                nc.sync.dma_start(out=x2t[:, kt], in_=x2s[kt])
            f3t = sm.tile([128, 4], dt.float32)
            nc.sync.dma_start(out=f3t[:], in_=f3[:])
            b3t = sm.tile([128, 4], dt.float32)
            nc.sync.dma_start(out=b3t[:], in_=b3[:])
            b2t = sm.tile([NC, 1], dt.float32)
            nc.sync.dma_start(out=b2t[:], in_=b2[:])
            w2t = sm.tile([128, 4, NC], dt.float32r)
            for kt in range(4):
                nc.sync.dma_start(out=w2t[:, kt], in_=w2[kt])
            h1 = big.tile([128, 4, 2304], dt.float32r)
            for gs in PASSES1:
                psums = {}
                for g in gs:
                    for mt in range(4):
                        psums[(g, mt)] = cp.tile([128, GROUPS1[g][1] * 96], dt.float32, tag="cps", name=f"cps_{g}_{mt}")
                for ci in range(72):
                    tap, kt = divmod(ci, 8)
                    dy, dx = divmod(tap, 3)
                    wbuf = wp.tile([128, 512], dt.float32r, tag="w")
                    nc.sync.dma_start(out=wbuf[:], in_=w1[ci])
                    for mt in range(4):
                        for g in gs:
                            r0, ng = GROUPS1[g]
                            rhs = x2t[:, kt, r0 + dy:r0 + dy + ng, dx:dx + 96]
                            nc.tensor.matmul(psums[(g, mt)][:], wbuf[:, mt * 128:(mt + 1) * 128],
                                             rhs, start=(ci == 0), stop=(ci == 71))
                for g in gs:
                    r0, ng = GROUPS1[g]
                    for mt in range(4):
                        nc.scalar.activation(h1[:, mt, r0 * 96:(r0 + ng) * 96], psums[(g, mt)][:],
                                             AF.Relu, bias=b3t[:, mt:mt + 1], scale=f3t[:, mt:mt + 1])
            aux2t = sm.tile([NC, 2304], dt.float32)
            for c0 in range(0, 2304, 512):
                cn = min(512, 2304 - c0)
                psA = cp.tile([NC, 512], dt.float32, tag="cps", name=f"psA_{c0}")
                for kt in range(4):
                    nc.tensor.matmul(psA[:, 0:cn], w2t[:, kt], h1[:, kt, c0:c0 + cn],
                                     start=(kt == 0), stop=(kt == 3))
                nc.vector.tensor_scalar_add(out=aux2t[:, c0:c0 + cn], in0=psA[:, 0:cn],
                                            scalar1=b2t[:, 0:1])
            nc.sync.dma_start(out=aux2_sh[:].rearrange("p r x -> p (r x)"), in_=aux2t[:])
    nc.compile()
    return nc


def _build_l2():
    nc = bacc.Bacc("TRN2", target_bir_lowering=False, num_devices=8)
    qs = nc.dram_tensor("qs", [64, 768], dt.float32r, kind="ExternalInput").ap()
    kf = nc.dram_tensor("kf", [64, 2304], dt.float32r, kind="ExternalInput").ap()
    ps = nc.dram_tensor("ps", [NC, 768], dt.float32r, kind="ExternalInput").ap()
    pf = nc.dram_tensor("pf", [NC, 2304], dt.float32r, kind="ExternalInput").ap()
    wt = nc.dram_tensor("wt", [18, 128, 512], dt.float32r, kind="ExternalInput").ap()
    wf = nc.dram_tensor("wf", [36, 128, 512], dt.float32r, kind="ExternalInput").ap()
    xs = nc.dram_tensor("xs", [4, 128, 16, 48], dt.float32, kind="ExternalInput").ap()
    ident = nc.dram_tensor("ident", [128, 128], dt.float32, kind="ExternalInput").ap()
    gmb = nc.dram_tensor("gmb", [128, 1], dt.float32, kind="ExternalInput").ap()
    wrow = nc.dram_tensor("wrow", [128, 78], dt.float32, kind="ExternalInput").ap()
    wcol = nc.dram_tensor("wcol", [128, 2, 4, 94], dt.float32, kind="ExternalInput").ap()
    ff = nc.dram_tensor("ff", [128, 4], dt.float32, kind="ExternalInput").ap()
    fb = nc.dram_tensor("fb", [128, 4], dt.float32, kind="ExternalInput").ap()
    zp = nc.dram_tensor("zp", [128, 4, 26, 2], dt.float32r, kind="ExternalInput").ap()
    o_sh = nc.dram_tensor("o_sh", [4, 128, 24, 96], dt.float32, kind="ExternalOutput").ap()
    with tile.TileContext(nc) as tc:
        with (
            tc.tile_pool(name="persist", bufs=1) as persist,
            tc.tile_pool(name="prep", bufs=1) as prep,
            tc.tile_pool(name="psm", bufs=8, space="PSUM") as psm,
        ):
            idt = persist.tile([128, 128], dt.float32)
            nc.sync.dma_start(out=idt[:], in_=ident[:])
            gmt = persist.tile([128, 1], dt.float32)
            nc.sync.dma_start(out=gmt[:], in_=gmb[:])
            wrt = persist.tile([128, 78], dt.float32)
            nc.sync.dma_start(out=wrt[:], in_=wrow[:])
            wct = persist.tile([128, 2, 4, 94], dt.float32)
            nc.sync.dma_start(out=wct[:], in_=wcol[:])
            fft = persist.tile([128, 4], dt.float32)
            nc.sync.dma_start(out=fft[:], in_=ff[:])
            fbt = persist.tile([128, 4], dt.float32)
            nc.sync.dma_start(out=fbt[:], in_=fb[:])
            xst = persist.tile([128, 4, 16, 48], dt.float32)
            for mt in range(4):
                nc.sync.dma_start(out=xst[:, mt], in_=xs[mt])
            pre = prep.tile([128, 4, 16, 48], dt.float32)

            with (
                tc.tile_pool(name="mid", bufs=1) as mid,
                tc.tile_pool(name="qk", bufs=1) as qk,
            ):
                qst = qk.tile([64, 768], dt.float32r)
                nc.sync.dma_start(out=qst[:], in_=qs[:])
                kft = qk.tile([64, 2304], dt.float32r)
                nc.sync.dma_start(out=kft[:], in_=kf[:])
                pst = qk.tile([NC, 768], dt.float32r)
                nc.sync.dma_start(out=pst[:], in_=ps[:])
                pft = qk.tile([NC, 2304], dt.float32r)
                nc.sync.dma_start(out=pft[:], in_=pf[:])
                wtt = mid.tile([128, 18, 512], dt.float32r)
                for mk in range(18):
                    nc.sync.dma_start(out=wtt[:, mk], in_=wt[mk])
                relT = mid.tile([128, 18, 768], dt.float32r)

                with tc.tile_pool(name="attw", bufs=2) as attw:
                    for nt in range(6):
                        erel = attw.tile([128, 2304], dt.float32, tag="erel")
                        gate = attw.tile([128, 2304], dt.float32, tag="gate")
                        for c0 in range(0, 2304, 512):
                            cn = min(512, 2304 - c0)
                            lg = psm.tile([128, 512], dt.float32, tag="ps", name=f"lg_{nt}_{c0}")
                            nc.tensor.matmul(lg[:, 0:cn], qst[:, nt * 128:(nt + 1) * 128],
                                             kft[:, c0:c0 + cn], start=True, stop=True)
                            nc.scalar.activation(erel[:, c0:c0 + cn], lg[:, 0:cn], AF.Exp)
                            gg = psm.tile([128, 512], dt.float32, tag="ps", name=f"gg_{nt}_{c0}")
                            nc.tensor.matmul(gg[:, 0:cn], pst[:, nt * 128:(nt + 1) * 128],
                                             pft[:, c0:c0 + cn], start=True, stop=True)
                            nc.scalar.activation(gate[:, c0:c0 + cn], gg[:, 0:cn], AF.Sigmoid)
                        rsum = attw.tile([128, 1], dt.float32, tag="rsum")
                        nc.vector.tensor_reduce(out=rsum[:], in_=erel[:], axis=mybir.AxisListType.X, op=OP.add)
                        rin = attw.tile([128, 1], dt.float32, tag="rin")
                        nc.vector.reciprocal(out=rin[:], in_=rsum[:])
                        nc.vector.scalar_tensor_tensor(out=erel[:], in0=erel[:], scalar=rin[:, 0:1],
                                                       in1=gate[:], op0=OP.mult, op1=OP.mult)
                        for mk in range(18):
                            tp = psm.tile([128, 128], dt.float32, tag="ps", name=f"tp_{nt}_{mk}")
                            nc.tensor.transpose(tp[:], erel[:, mk * 128:(mk + 1) * 128], idt[:])
                            nc.scalar.copy(out=relT[:, mk, nt * 128:(nt + 1) * 128], in_=tp[:])

                for mt in range(4):
                    aga = psm.tile([128, 512], dt.float32, tag="ps", name=f"aga_{mt}")
                    agb = psm.tile([128, 256], dt.float32, tag="ps", name=f"agb_{mt}")
                    for mk in range(18):
                        nc.tensor.matmul(aga[:], wtt[:, mk, mt * 128:(mt + 1) * 128],
                                         relT[:, mk, 0:512], start=(mk == 0), stop=(mk == 17))
                        nc.tensor.matmul(agb[:], wtt[:, mk, mt * 128:(mt + 1) * 128],
                                         relT[:, mk, 512:768], start=(mk == 0), stop=(mk == 17))
                    nc.vector.scalar_tensor_tensor(
                        out=pre[:, mt].rearrange("p r x -> p (r x)")[:, 0:512],
                        in0=aga[:], scalar=gmt[:, 0:1],
                        in1=xst[:, mt].rearrange("p r x -> p (r x)")[:, 0:512],
                        op0=OP.mult, op1=OP.add)
                    nc.vector.scalar_tensor_tensor(
                        out=pre[:, mt].rearrange("p r x -> p (r x)")[:, 512:768],
                        in0=agb[:], scalar=gmt[:, 0:1],
                        in1=xst[:, mt].rearrange("p r x -> p (r x)")[:, 512:768],
                        op0=OP.mult, op1=OP.add)

            with (
                tc.tile_pool(name="late", bufs=1) as late,
                tc.tile_pool(name="wps", bufs=4) as wps,
                tc.tile_pool(name="stg", bufs=2) as stg,
                tc.tile_pool(name="upw", bufs=2) as upw,
            ):
                rmix = late.tile([128, 4, 26, 48], dt.float32)
                for jj in range(26):
                    q0 = jj // 2 + 1
                    t0 = upw.tile([128, 4, 48], dt.float32, tag="t0")
                    nc.vector.tensor_scalar(out=t0[:], in0=pre[:, :, q0, :],
                                            scalar1=wrt[:, 3 * jj:3 * jj + 1], scalar2=0.0,
                                            op0=OP.mult, op1=OP.bypass)
                    t1 = upw.tile([128, 4, 48], dt.float32, tag="t1")
                    nc.vector.scalar_tensor_tensor(out=t1[:], in0=pre[:, :, q0 + 1, :],
                                                   scalar=wrt[:, 3 * jj + 1:3 * jj + 2], in1=t0[:],
                                                   op0=OP.mult, op1=OP.add)
                    nc.vector.scalar_tensor_tensor(out=rmix[:, :, jj, :], in0=pre[:, :, q0 + 2, :],
                                                   scalar=wrt[:, 3 * jj + 2:3 * jj + 3], in1=t1[:],
                                                   op0=OP.mult, op1=OP.add)
                upc = late.tile([128, 4, 26, 98], dt.float32r)
                nc.sync.dma_start(out=upc[:, :, :, 0:1], in_=zp[:, :, :, 0:1])
                nc.sync.dma_start(out=upc[:, :, :, 97:98], in_=zp[:, :, :, 1:2])
                for jj in range(26):
                    srcA = rmix[:, :, jj, 0:47].broadcast_to((128, 4, 47, 2))
                    srcB = rmix[:, :, jj, 1:48].broadcast_to((128, 4, 47, 2))
                    ta = upw.tile([128, 4, 94], dt.float32, tag="ta")
                    nc.vector.tensor_tensor(out=ta[:].rearrange("p m (u two) -> p m u two", two=2),
                                            in0=srcA, in1=wct[:, 0].rearrange("p m (u two) -> p m u two", two=2),
                                            op=OP.mult)
                    nc.vector.tensor_tensor(out=upc[:, :, jj, 2:96].rearrange("p m (u two) -> p m u two", two=2),
                                            in0=srcB, in1=wct[:, 1].rearrange("p m (u two) -> p m u two", two=2),
                                            op=OP.mult)
                    nc.vector.tensor_tensor(out=upc[:, :, jj, 2:96], in0=upc[:, :, jj, 2:96],
                                            in1=ta[:], op=OP.add)
                    nc.vector.tensor_copy(out=upc[:, :, jj, 1:2], in_=rmix[:, :, jj, 0:1])
                    nc.vector.tensor_copy(out=upc[:, :, jj, 96:97], in_=rmix[:, :, jj, 47:48])
                for g in range(5):
                    r0, ng = GROUPS2[g]
                    cps = {}
                    for mt in range(4):
                        cps[mt] = psm.tile([128, ng * 96], dt.float32, tag="ps", name=f"c2_{g}_{mt}")
                    for ci in range(36):
                        tap, kt = divmod(ci, 4)
                        dy, dx = divmod(tap, 3)
                        wbuf = wps.tile([128, 512], dt.float32r, tag="wf")
                        nc.sync.dma_start(out=wbuf[:], in_=wf[ci])
                        for mt in range(4):
                            rhs = upc[:, kt, r0 + dy - 1:r0 + dy - 1 + ng, dx:dx + 96]
                            nc.tensor.matmul(cps[mt][:], wbuf[:, mt * 128:(mt + 1) * 128], rhs,
                                             start=(ci == 0), stop=(ci == 35))
                    for mt in range(4):
                        so = stg.tile([128, 480], dt.float32, tag="so")
                        nc.scalar.activation(so[:, 0:ng * 96], cps[mt][:], AF.Relu,
                                             bias=fbt[:, mt:mt + 1], scale=fft[:, mt:mt + 1])
                        nc.sync.dma_start(out=o_sh[mt, :, r0 - 1:r0 - 1 + ng, :],
                                          in_=so[:, 0:ng * 96].rearrange("p (r x) -> p r x", x=96))
    nc.compile()
    return nc


_L1 = None
_L2 = None


def _shard_rows(i):
    return 12 * i - 2  # local row 0 of the 16-row query shard


def kernel(x, x2, w_q, w_k, w_v, w_o3_1, bn3_s, bn3_b, w_o3_2, b_o3_2,
           gamma, w_f, bnf_s, bnf_b):
    global _L1, _L2
    x = np.asarray(x, np.float32)
    x2 = np.asarray(x2, np.float32)
    w_q = np.asarray(w_q, np.float32); w_k = np.asarray(w_k, np.float32)
    w_v = np.asarray(w_v, np.float32)
    w_o3_1 = np.asarray(w_o3_1, np.float32)
    w_o3_2 = np.asarray(w_o3_2, np.float32)
    gamma = np.asarray(gamma, np.float32); w_f = np.asarray(w_f, np.float32)
    f3v = (np.asarray(bn3_s, np.float32) / np.sqrt(np.float32(1.0 + EPS)))
    b3v = np.asarray(bn3_b, np.float32)
    ffv = (np.asarray(bnf_s, np.float32) / np.sqrt(np.float32(1.0 + EPS)))
    fbv = np.asarray(bnf_b, np.float32)

    if _L1 is None:
        _L1 = _build_l1()
    if _L2 is None:
        _L2 = _build_l2()

    # ---- launch 1: conv_out3 ----
    # weights: [72][ci 128][co 512] ; chunk ci index = tap*8+kt, tap = dy*3+dx
    w1_arr = np.ascontiguousarray(
        w_o3_1.transpose(2, 3, 1, 0).reshape(9, 8, 128, 512).reshape(72, 128, 512))
    w2_arr = np.ascontiguousarray(w_o3_2.T.reshape(4, 128, NC))
    f3_arr = np.ascontiguousarray(f3v.reshape(4, 128).T)
    b3_arr = np.ascontiguousarray(b3v.reshape(4, 128).T)
    b2_arr = np.asarray(b_o3_2, np.float32).reshape(NC, 1)
    x2p = np.zeros((B, 1024, 98, 98), np.float32)
    x2p[:, :, 1:97, 1:97] = x2
    in1 = []
    for core in CORES:
        g, i = divmod(core, 4)
        rows = x2p[g, :, 24 * i:24 * i + 26, :]  # padded rows 24i-1..24i+25
        in1.append({
            "x2s": np.ascontiguousarray(rows.reshape(8, 128, 26, 98)),
            "w1": w1_arr, "w2": w2_arr, "f3": f3_arr, "b3": b3_arr, "b2": b2_arr,
        })
    r1 = run_bass_kernel_spmd(_L1, in1, CORES, trace=PROFILE)
    aux2 = np.zeros((B, NC, 96, 96), np.float32)
    for core in CORES:
        g, i = divmod(core, 4)
        aux2[g, :, 24 * i:24 * i + 24, :] = r1.results[core]["aux2_sh"]

    # ---- host glue ----
    # bilinear downsample 96->48 (align_corners) + channel softmax -> pred
    def bil(a, oh, ow):
        Bc, Cc, Hh, Ww = a.shape
        ys = np.linspace(0.0, Hh - 1.0, oh); xsx = np.linspace(0.0, Ww - 1.0, ow)
        y0 = np.floor(ys).astype(int); y1 = np.minimum(y0 + 1, Hh - 1); wy = (ys - y0).astype(np.float32)
        x0 = np.floor(xsx).astype(int); x1 = np.minimum(x0 + 1, Ww - 1); wx = (xsx - x0).astype(np.float32)
        r0 = a[:, :, y0, :]; rr1 = a[:, :, y1, :]
        r = r0 * (1 - wy)[None, None, :, None] + rr1 * wy[None, None, :, None]
        c0 = r[:, :, :, x0]; c1 = r[:, :, :, x1]
        return c0 * (1 - wx)[None, None, None, :] + c1 * wx[None, None, None, :]

    a48 = bil(aux2, 48, 48)
    a48 = a48 - a48.max(axis=1, keepdims=True)
    e = np.exp(a48)
    pred = (e / e.sum(axis=1, keepdims=True)).reshape(B, NC, N).astype(np.float32)

    xf = x.reshape(B, C, N)
    q = np.ascontiguousarray(np.einsum('oc,bcn->bon', w_q, xf, optimize=True), np.float32)
    k = np.ascontiguousarray(np.einsum('oc,bcn->bon', w_k, xf, optimize=True), np.float32)
    v = np.ascontiguousarray(np.einsum('oc,bcn->bon', w_v, xf, optimize=True), np.float32)

    # lr[b,j,n] = sigmoid(sum_c q * shift_j(k))
    kp = np.zeros((B, 64, 50, 50), np.float32)
    kp[:, :, 1:49, 1:49] = k.reshape(B, 64, H, W)
    q4 = q.reshape(B, 64, H, W)
    lr = np.zeros((B, 9, H, W), np.float32)
    for dy in range(3):
        for dx in range(3):
            lr[:, dy * 3 + dx] = (q4 * kp[:, :, dy:dy + H, dx:dx + W]).sum(axis=1)
    lr = (1.0 / (1.0 + np.exp(-lr)))

    # W[b,c,y+dy-1,x+dx-1] += (lr_j * v)[b,c,y,x]
    v4 = v.reshape(B, C, H, W)
    Wm = np.zeros((B, C, H, W), np.float32)
    for dy in range(3):
        for dx in range(3):
            lv = lr[:, dy * 3 + dx][:, None] * v4
            sy0, sy1 = max(0, 1 - dy), min(H, H + 1 - dy)
            sx0, sx1 = max(0, 1 - dx), min(W, W + 1 - dx)
            Wm[:, :, sy0 + dy - 1:sy1 + dy - 1, sx0 + dx - 1:sx1 + dx - 1] += lv[:, :, sy0:sy1, sx0:sx1]
    WT = Wm.reshape(B, C, N).transpose(0, 2, 1)  # [B, 2304, 512]
    wt_arrs = [np.ascontiguousarray(WT[g].reshape(18, 128, 512)) for g in range(B)]

    wf_arr = np.ascontiguousarray(
        w_f.transpose(2, 3, 1, 0).reshape(9, 4, 128, 512).reshape(36, 128, 512))
    ff_arr = np.ascontiguousarray(ffv.reshape(4, 128).T)
    fb_arr = np.ascontiguousarray(fbv.reshape(4, 128).T)
    ident = np.eye(128, dtype=np.float32)
    gmb = np.full((128, 1), float(gamma.reshape(-1)[0]), np.float32)

    # upsample col-mix constants (uniform)
    wc = np.zeros((2, 94), np.float32)
    for jx in range(1, 95):
        xx = jx * 47.0 / 95.0
        x0 = int(np.floor(xx)); wv_ = xx - x0
        u = (jx - 1) // 2
        assert x0 == u, (jx, x0, u)
        wc[0, jx - 1] = 1.0 - wv_
        wc[1, jx - 1] = wv_
    wcol_arr = np.ascontiguousarray(np.broadcast_to(wc[None, :, None, :], (128, 2, 4, 94)))

    in2 = []
    for core in CORES:
        g, i = divmod(core, 4)
        lo = _shard_rows(i)
        # query shard rows lo..lo+16 (clipped, zero pad)
        def shard(a2d, P):  # a2d [P, H, W] -> [P, 16, 48]
            out = np.zeros((P, 16, 48), np.float32)
            y0 = max(0, lo); y1 = min(H, lo + 16)
            out[:, y0 - lo:y1 - lo, :] = a2d[:, y0:y1, :]
            return out
        qsh = shard(q[g].reshape(64, H, W), 64).reshape(64, 768)
        psh = shard(pred[g].reshape(NC, H, W), NC).reshape(NC, 768)
        xsh = shard(x[g].reshape(C, H, W), C).reshape(4, 128, 16, 48)
        # row-mix weights
        wr = np.zeros((26, 3), np.float32)
        for jj in range(26):
            j = 24 * i - 1 + jj
            if 0 <= j < 96:
                yy = j * 47.0 / 95.0
                y0 = int(np.floor(yy)); wv_ = yy - y0
                y0r = y0 - lo
                q0 = jj // 2 + 1
                d = y0r - q0
                assert d in (0, 1), (i, jj, d)
                wr[jj, d] += 1.0 - wv_
                wr[jj, d + 1] += wv_
        wrow_arr = np.ascontiguousarray(np.broadcast_to(wr.reshape(1, 78), (128, 78)))
        in2.append({
            "qs": np.ascontiguousarray(qsh), "kf": np.ascontiguousarray(k[g]),
            "ps": np.ascontiguousarray(psh), "pf": np.ascontiguousarray(pred[g]),
            "wt": wt_arrs[g], "wf": wf_arr, "xs": np.ascontiguousarray(xsh),
            "ident": ident, "gmb": gmb, "wrow": wrow_arr, "wcol": wcol_arr,
            "zp": np.zeros((128, 4, 26, 2), np.float32),
            "ff": ff_arr, "fb": fb_arr,
        })
    r2 = run_bass_kernel_spmd(_L2, in2, CORES, trace=PROFILE)
    out = np.zeros((B, C, 96, 96), np.float32)
    for core in CORES:
        g, i = divmod(core, 4)
        o = r2.results[core]["o_sh"]  # [4,128,24,96]
        out[g, :, 24 * i:24 * i + 24, :] = o.reshape(512, 24, 96)

    kernel.last_exec_ns = (r1.exec_time_ns or 0) + (r2.exec_time_ns or 0) if PROFILE else None
    return out, aux2


# revision 9
# speedup vs baseline: 1.1573x; 1.1573x over previous
import numpy as np
import concourse.bacc as bacc
import concourse.mybir as mybir
from concourse import tile
from concourse.bass_utils import run_bass_kernel_spmd

dt = mybir.dt
AF = mybir.ActivationFunctionType
OP = mybir.AluOpType

EPS = 1e-5
B, C, H, W = 2, 512, 48, 48
N = H * W
NC = 19
CORES = list(range(8))
PROFILE = False

GROUPS1 = [(0, 5), (5, 5), (10, 5), (15, 5), (20, 4)]   # conv output row groups (local)
PASSES1 = [(0, 1), (2, 3), (4,)]
GROUPS2 = [(1, 5), (6, 5), (11, 5), (16, 5), (21, 4)]   # conv2 out rows in local up coords


def _build_l1():
    nc = bacc.Bacc("TRN2", target_bir_lowering=False, num_devices=8)
    x2s = nc.dram_tensor("x2s", [8, 128, 26, 98], dt.float32r, kind="ExternalInput").ap()
    w1 = nc.dram_tensor("w1", [72, 128, 512], dt.float32r, kind="ExternalInput").ap()
    w2 = nc.dram_tensor("w2", [4, 128, NC], dt.float32r, kind="ExternalInput").ap()
    f3 = nc.dram_tensor("f3", [128, 4], dt.float32, kind="ExternalInput").ap()
    b3 = nc.dram_tensor("b3", [128, 4], dt.float32, kind="ExternalInput").ap()
    b2 = nc.dram_tensor("b2", [NC, 1], dt.float32, kind="ExternalInput").ap()
    aux2_sh = nc.dram_tensor("aux2_sh", [NC, 24, 96], dt.float32, kind="ExternalOutput").ap()
    with tile.TileContext(nc) as tc:
        with (
            tc.tile_pool(name="big", bufs=1) as big,
            tc.tile_pool(name="wp", bufs=8) as wp,
            tc.tile_pool(name="cp", bufs=8, space="PSUM") as cp,
            tc.tile_pool(name="sm", bufs=1) as sm,
        ):
            x2t = big.tile([128, 8, 26, 98], dt.float32r)
            for kt in range(8):
                nc.sync.dma_start(out=x2t[:, kt], in_=x2s[kt])
            f3t = sm.tile([128, 4], dt.float32)
            nc.sync.dma_start(out=f3t[:], in_=f3[:])
            b3t = sm.tile([128, 4], dt.float32)
            nc.sync.dma_start(out=b3t[:], in_=b3[:])
            b2t = sm.tile([NC, 1], dt.float32)
            nc.sync.dma_start(out=b2t[:], in_=b2[:])
            w2t = sm.tile([128, 4, NC], dt.float32r)
            for kt in range(4):
                nc.sync.dma_start(out=w2t[:, kt], in_=w2[kt])
            h1 = big.tile([128, 4, 2304], dt.float32r)
            for gs in PASSES1:
                psums = {}
                for g in gs:
                    for mt in range(4):
                        psums[(g, mt)] = cp.tile([128, GROUPS1[g][1] * 96], dt.float32, tag="cps", name=f"cps_{g}_{mt}")
                for ci in range(72):
                    tap, kt = divmod(ci, 8)
                    dy, dx = divmod(tap, 3)
                    wbuf = wp.tile([128, 512], dt.float32r, tag="w")
                    nc.sync.dma_start(out=wbuf[:], in_=w1[ci])
                    for mt in range(4):
                        for g in gs:
                            r0, ng = GROUPS1[g]
                            rhs = x2t[:, kt, r0 + dy:r0 + dy + ng, dx:dx + 96]
                            nc.tensor.matmul(psums[(g, mt)][:], wbuf[:, mt * 128:(mt + 1) * 128],
                                             rhs, start=(ci == 0), stop=(ci == 71))
                for g in gs:
                    r0, ng = GROUPS1[g]
                    for mt in range(4):
                        nc.scalar.activation(h1[:, mt, r0 * 96:(r0 + ng) * 96], psums[(g, mt)][:],
                                             AF.Relu, bias=b3t[:, mt:mt + 1], scale=f3t[:, mt:mt + 1])
            aux2t = sm.tile([NC, 2304], dt.float32)
            for c0 in range(0, 2304, 512):
                cn = min(512, 2304 - c0)
                psA = cp.tile([NC, 512], dt.float32, tag="cps", name=f"psA_{c0}")
                for kt in range(4):
                    nc.tensor.matmul(psA[:, 0:cn], w2t[:, kt], h1[:, kt, c0:c0 + cn],
                                     start=(kt == 0), stop=(kt == 3))
                nc.vector.tensor_scalar_add(out=aux2t[:, c0:c0 + cn], in0=psA[:, 0:cn],
                                            scalar1=b2t[:, 0:1])
            nc.sync.dma_start(out=aux2_sh[:].rearrange("p r x -> p (r x)"), in_=aux2t[:])
    nc.compile()
    return nc


def _build_l2():
    nc = bacc.Bacc("TRN2", target_bir_lowering=False, num_devices=8)
    qs = nc.dram_tensor("qs", [64, 768], dt.float32r, kind="ExternalInput").ap()
    kf = nc.dram_tensor("kf", [64, 2304], dt.float32r, kind="ExternalInput").ap()
    ps = nc.dram_tensor("ps", [NC, 768], dt.float32r, kind="ExternalInput").ap()
    pf = nc.dram_tensor("pf", [NC, 2304], dt.float32r, kind="ExternalInput").ap()
    wt = nc.dram_tensor("wt", [18, 128, 512], dt.float32r, kind="ExternalInput").ap()
    wf = nc.dram_tensor("wf", [36, 128, 512], dt.float32r, kind="ExternalInput").ap()
    xs = nc.dram_tensor("xs", [4, 128, 16, 48], dt.float32, kind="ExternalInput").ap()
    ident = nc.dram_tensor("ident", [128, 128], dt.float32, kind="ExternalInput").ap()
    gmb = nc.dram_tensor("gmb", [128, 1], dt.float32, kind="ExternalInput").ap()
    wrow = nc.dram_tensor("wrow", [128, 78], dt.float32, kind="ExternalInput").ap()
    wcol = nc.dram_tensor("wcol", [128, 2, 4, 94], dt.float32, kind="ExternalInput").ap()
    ff = nc.dram_tensor("ff", [128, 4], dt.float32, kind="ExternalInput").ap()
    fb = nc.dram_tensor("fb", [128, 4], dt.float32, kind="ExternalInput").ap()
    zp = nc.dram_tensor("zp", [128, 4, 26, 2], dt.float32r, kind="ExternalInput").ap()
    o_sh = nc.dram_tensor("o_sh", [4, 128, 24, 96], dt.float32, kind="ExternalOutput").ap()
    with tile.TileContext(nc) as tc:
        with (
            tc.tile_pool(name="persist", bufs=1) as persist,
            tc.tile_pool(name="prep", bufs=1) as prep,
            tc.tile_pool(name="psm", bufs=8, space="PSUM") as psm,
        ):
            idt = persist.tile([128, 128], dt.float32)
            nc.sync.dma_start(out=idt[:], in_=ident[:])
            gmt = persist.tile([128, 1], dt.float32)
            nc.sync.dma_start(out=gmt[:], in_=gmb[:])
            wrt = persist.tile([128, 78], dt.float32)
            nc.sync.dma_start(out=wrt[:], in_=wrow[:])
            wct = persist.tile([128, 2, 4, 94], dt.float32)
            nc.sync.dma_start(out=wct[:], in_=wcol[:])
            fft = persist.tile([128, 4], dt.float32)
            nc.sync.dma_start(out=fft[:], in_=ff[:])
            fbt = persist.tile([128, 4], dt.float32)
            nc.sync.dma_start(out=fbt[:], in_=fb[:])
            xst = persist.tile([128, 4, 16, 48], dt.float32)
            for mt in range(4):
                nc.sync.dma_start(out=xst[:, mt], in_=xs[mt])
            pre = prep.tile([128, 4, 16, 48], dt.float32)

            with (
                tc.tile_pool(name="mid", bufs=1) as mid,
                tc.tile_pool(name="qk", bufs=1) as qk,
            ):
                qst = qk.tile([64, 768], dt.float32r)
                nc.sync.dma_start(out=qst[:], in_=qs[:])
                kft = qk.tile([64, 2304], dt.float32r)
                nc.sync.dma_start(out=kft[:], in_=kf[:])
                pst = qk.tile([NC, 768], dt.float32r)
                nc.sync.dma_start(out=pst[:], in_=ps[:])
                pft = qk.tile([NC, 2304], dt.float32r)
                nc.sync.dma_start(out=pft[:], in_=pf[:])
                wtt = mid.tile([128, 18, 512], dt.float32r)
                for mk in range(18):
                    nc.sync.dma_start(out=wtt[:, mk], in_=wt[mk])
                relT = mid.tile([128, 18, 768], dt.float32r)

                with tc.tile_pool(name="attw", bufs=2) as attw:
                    for nt in range(6):
                        erel = attw.tile([128, 2304], dt.float32, tag="erel")
                        gate = attw.tile([128, 2304], dt.float32, tag="gate")
                        racc = attw.tile([128, 5], dt.float32, tag="racc")
                        for ic, c0 in enumerate(range(0, 2304, 512)):
                            cn = min(512, 2304 - c0)
                            lg = psm.tile([128, 512], dt.float32, tag="ps", name=f"lg_{nt}_{c0}")
                            nc.tensor.matmul(lg[:, 0:cn], qst[:, nt * 128:(nt + 1) * 128],
                                             kft[:, c0:c0 + cn], start=True, stop=True)
                            nc.scalar.activation(erel[:, c0:c0 + cn], lg[:, 0:cn], AF.Exp,
                                                 accum_out=racc[:, ic:ic + 1])
                            gg = psm.tile([128, 512], dt.float32, tag="ps", name=f"gg_{nt}_{c0}")
                            nc.tensor.matmul(gg[:, 0:cn], pst[:, nt * 128:(nt + 1) * 128],
                                             pft[:, c0:c0 + cn], start=True, stop=True)
                            nc.scalar.activation(gate[:, c0:c0 + cn], gg[:, 0:cn], AF.Sigmoid)
                        rsum = attw.tile([128, 1], dt.float32, tag="rsum")
                        nc.vector.tensor_reduce(out=rsum[:], in_=racc[:], axis=mybir.AxisListType.X, op=OP.add)
                        rin = attw.tile([128, 1], dt.float32, tag="rin")
                        nc.vector.reciprocal(out=rin[:], in_=rsum[:])
                        nc.vector.scalar_tensor_tensor(out=erel[:], in0=erel[:], scalar=rin[:, 0:1],
                                                       in1=gate[:], op0=OP.mult, op1=OP.mult)
                        for mk in range(18):
                            tp = psm.tile([128, 128], dt.float32, tag="ps", name=f"tp_{nt}_{mk}")
                            nc.tensor.transpose(tp[:], erel[:, mk * 128:(mk + 1) * 128], idt[:])
                            nc.scalar.copy(out=relT[:, mk, nt * 128:(nt + 1) * 128], in_=tp[:])

                for mt in range(4):
                    aga = psm.tile([128, 512], dt.float32, tag="ps", name=f"aga_{mt}")
                    agb = psm.tile([128, 256], dt.float32, tag="ps", name=f"agb_{mt}")
                    for mk in range(18):
                        nc.tensor.matmul(aga[:], wtt[:, mk, mt * 128:(mt + 1) * 128],
                                         relT[:, mk, 0:512], start=(mk == 0), stop=(mk == 17))
                        nc.tensor.matmul(agb[:], wtt[:, mk, mt * 128:(mt + 1) * 128],
                                         relT[:, mk, 512:768], start=(mk == 0), stop=(mk == 17))
                    nc.vector.scalar_tensor_tensor(
                        out=pre[:, mt].rearrange("p r x -> p (r x)")[:, 0:512],
                        in0=aga[:], scalar=gmt[:, 0:1],
                        in1=xst[:, mt].rearrange("p r x -> p (r x)")[:, 0:512],
                        op0=OP.mult, op1=OP.add)
                    nc.vector.scalar_tensor_tensor(
                        out=pre[:, mt].rearrange("p r x -> p (r x)")[:, 512:768],
                        in0=agb[:], scalar=gmt[:, 0:1],
                        in1=xst[:, mt].rearrange("p r x -> p (r x)")[:, 512:768],
                        op0=OP.mult, op1=OP.add)

            with (
                tc.tile_pool(name="late", bufs=1) as late,
                tc.tile_pool(name="wps", bufs=8) as wps,
                tc.tile_pool(name="stg", bufs=2) as stg,
                tc.tile_pool(name="upw", bufs=2) as upw,
            ):
                wft = late.tile([128, 36, 512], dt.float32r)
                for ci in range(36):
                    nc.sync.dma_start(out=wft[:, ci], in_=wf[ci])
                rmix = late.tile([128, 4, 26, 48], dt.float32)
                for jj in range(26):
                    q0 = jj // 2 + 1
                    t0 = upw.tile([128, 4, 48], dt.float32, tag="t0")
                    nc.vector.tensor_scalar(out=t0[:], in0=pre[:, :, q0, :],
                                            scalar1=wrt[:, 3 * jj:3 * jj + 1], scalar2=0.0,
                                            op0=OP.mult, op1=OP.bypass)
                    t1 = upw.tile([128, 4, 48], dt.float32, tag="t1")
                    nc.vector.scalar_tensor_tensor(out=t1[:], in0=pre[:, :, q0 + 1, :],
                                                   scalar=wrt[:, 3 * jj + 1:3 * jj + 2], in1=t0[:],
                                                   op0=OP.mult, op1=OP.add)
                    nc.vector.scalar_tensor_tensor(out=rmix[:, :, jj, :], in0=pre[:, :, q0 + 2, :],
                                                   scalar=wrt[:, 3 * jj + 2:3 * jj + 3], in1=t1[:],
                                                   op0=OP.mult, op1=OP.add)
                upc = late.tile([128, 4, 26, 98], dt.float32r)
                nc.sync.dma_start(out=upc[:, :, :, 0:1], in_=zp[:, :, :, 0:1])
                nc.sync.dma_start(out=upc[:, :, :, 97:98], in_=zp[:, :, :, 1:2])
                for jj in range(26):
                    srcA = rmix[:, :, jj, 0:47].broadcast_to((128, 4, 47, 2))
                    srcB = rmix[:, :, jj, 1:48].broadcast_to((128, 4, 47, 2))
                    ta = upw.tile([128, 4, 94], dt.float32, tag="ta")
                    nc.vector.tensor_tensor(out=ta[:].rearrange("p m (u two) -> p m u two", two=2),
                                            in0=srcA, in1=wct[:, 0].rearrange("p m (u two) -> p m u two", two=2),
                                            op=OP.mult)
                    nc.vector.tensor_tensor(out=upc[:, :, jj, 2:96].rearrange("p m (u two) -> p m u two", two=2),
                                            in0=srcB, in1=wct[:, 1].rearrange("p m (u two) -> p m u two", two=2),
                                            op=OP.mult)
                    nc.vector.tensor_tensor(out=upc[:, :, jj, 2:96], in0=upc[:, :, jj, 2:96],
                                            in1=ta[:], op=OP.add)
                    nc.vector.tensor_copy(out=upc[:, :, jj, 1:2], in_=rmix[:, :, jj, 0:1])
                    nc.vector.tensor_copy(out=upc[:, :, jj, 96:97], in_=rmix[:, :, jj, 47:48])
                for g in range(5):
                    r0, ng = GROUPS2[g]
                    cps = {}
                    for mt in range(4):
                        cps[mt] = psm.tile([128, ng * 96], dt.float32, tag="ps", name=f"c2_{g}_{mt}")
                    for ci in range(36):
                        tap, kt = divmod(ci, 4)
                        dy, dx = divmod(tap, 3)
                        for mt in range(4):
                            rhs = upc[:, kt, r0 + dy - 1:r0 + dy - 1 + ng, dx:dx + 96]
                            nc.tensor.matmul(cps[mt][:], wft[:, ci, mt * 128:(mt + 1) * 128], rhs,
                                             start=(ci == 0), stop=(ci == 35))
                    for mt in range(4):
                        so = stg.tile([128, 480], dt.float32, tag="so")
                        nc.scalar.activation(so[:, 0:ng * 96], cps[mt][:], AF.Relu,
                                             bias=fbt[:, mt:mt + 1], scale=fft[:, mt:mt + 1])
                        nc.sync.dma_start(out=o_sh[mt, :, r0 - 1:r0 - 1 + ng, :],
                                          in_=so[:, 0:ng * 96].rearrange("p (r x) -> p r x", x=96))
    nc.compile()
    return nc


_L1 = None
_L2 = None


def _shard_rows(i):
    return 12 * i - 2  # local row 0 of the 16-row query shard


def kernel(x, x2, w_q, w_k, w_v, w_o3_1, bn3_s, bn3_b, w_o3_2, b_o3_2,
           gamma, w_f, bnf_s, bnf_b):
    global _L1, _L2
    x = np.asarray(x, np.float32)
    x2 = np.asarray(x2, np.float32)
    w_q = np.asarray(w_q, np.float32); w_k = np.asarray(w_k, np.float32)
    w_v = np.asarray(w_v, np.float32)
    w_o3_1 = np.asarray(w_o3_1, np.float32)
    w_o3_2 = np.asarray(w_o3_2, np.float32)
    gamma = np.asarray(gamma, np.float32); w_f = np.asarray(w_f, np.float32)
    f3v = (np.asarray(bn3_s, np.float32) / np.sqrt(np.float32(1.0 + EPS)))
    b3v = np.asarray(bn3_b, np.float32)
    ffv = (np.asarray(bnf_s, np.float32) / np.sqrt(np.float32(1.0 + EPS)))
    fbv = np.asarray(bnf_b, np.float32)

    if _L1 is None:
        _L1 = _build_l1()
    if _L2 is None:
        _L2 = _build_l2()

    # ---- launch 1: conv_out3 ----
    # weights: [72][ci 128][co 512] ; chunk ci index = tap*8+kt, tap = dy*3+dx
    w1_arr = np.ascontiguousarray(
        w_o3_1.transpose(2, 3, 1, 0).reshape(9, 8, 128, 512).reshape(72, 128, 512))
    w2_arr = np.ascontiguousarray(w_o3_2.T.reshape(4, 128, NC))
    f3_arr = np.ascontiguousarray(f3v.reshape(4, 128).T)
    b3_arr = np.ascontiguousarray(b3v.reshape(4, 128).T)
    b2_arr = np.asarray(b_o3_2, np.float32).reshape(NC, 1)
    x2p = np.zeros((B, 1024, 98, 98), np.float32)
    x2p[:, :, 1:97, 1:97] = x2
    in1 = []
    for core in CORES:
        g, i = divmod(core, 4)
        rows = x2p[g, :, 24 * i:24 * i + 26, :]  # padded rows 24i-1..24i+25
        in1.append({
            "x2s": np.ascontiguousarray(rows.reshape(8, 128, 26, 98)),
            "w1": w1_arr, "w2": w2_arr, "f3": f3_arr, "b3": b3_arr, "b2": b2_arr,
        })
    r1 = run_bass_kernel_spmd(_L1, in1, CORES, trace=PROFILE)
    aux2 = np.zeros((B, NC, 96, 96), np.float32)
    for core in CORES:
        g, i = divmod(core, 4)
        aux2[g, :, 24 * i:24 * i + 24, :] = r1.results[core]["aux2_sh"]

    # ---- host glue ----
    # bilinear downsample 96->48 (align_corners) + channel softmax -> pred
    def bil(a, oh, ow):
        Bc, Cc, Hh, Ww = a.shape
        ys = np.linspace(0.0, Hh - 1.0, oh); xsx = np.linspace(0.0, Ww - 1.0, ow)
        y0 = np.floor(ys).astype(int); y1 = np.minimum(y0 + 1, Hh - 1); wy = (ys - y0).astype(np.float32)
        x0 = np.floor(xsx).astype(int); x1 = np.minimum(x0 + 1, Ww - 1); wx = (xsx - x0).astype(np.float32)
        r0 = a[:, :, y0, :]; rr1 = a[:, :, y1, :]
        r = r0 * (1 - wy)[None, None, :, None] + rr1 * wy[None, None, :, None]
        c0 = r[:, :, :, x0]; c1 = r[:, :, :, x1]
        return c0 * (1 - wx)[None, None, None, :] + c1 * wx[None, None, None, :]

    a48 = bil(aux2, 48, 48)
    a48 = a48 - a48.max(axis=1, keepdims=True)
    e = np.exp(a48)
    pred = (e / e.sum(axis=1, keepdims=True)).reshape(B, NC, N).astype(np.float32)

    xf = x.reshape(B, C, N)
    q = np.ascontiguousarray(np.einsum('oc,bcn->bon', w_q, xf, optimize=True), np.float32)
    k = np.ascontiguousarray(np.einsum('oc,bcn->bon', w_k, xf, optimize=True), np.float32)
    v = np.ascontiguousarray(np.einsum('oc,bcn->bon', w_v, xf, optimize=True), np.float32)

    # lr[b,j,n] = sigmoid(sum_c q * shift_j(k))
    kp = np.zeros((B, 64, 50, 50), np.float32)
    kp[:, :, 1:49, 1:49] = k.reshape(B, 64, H, W)
    q4 = q.reshape(B, 64, H, W)
    lr = np.zeros((B, 9, H, W), np.float32)
    for dy in range(3):
        for dx in range(3):
            lr[:, dy * 3 + dx] = (q4 * kp[:, :, dy:dy + H, dx:dx + W]).sum(axis=1)
    lr = (1.0 / (1.0 + np.exp(-lr)))

    # W[b,c,y+dy-1,x+dx-1] += (lr_j * v)[b,c,y,x]
    v4 = v.reshape(B, C, H, W)
    Wm = np.zeros((B, C, H, W), np.float32)
    for dy in range(3):
        for dx in range(3):
            lv = lr[:, dy * 3 + dx][:, None] * v4
            sy0, sy1 = max(0, 1 - dy), min(H, H + 1 - dy)
            sx0, sx1 = max(0, 1 - dx), min(W, W + 1 - dx)
            Wm[:, :, sy0 + dy - 1:sy1 + dy - 1, sx0 + dx - 1:sx1 + dx - 1] += lv[:, :, sy0:sy1, sx0:sx1]
    WT = Wm.reshape(B, C, N).transpose(0, 2, 1)  # [B, 2304, 512]
    wt_arrs = [np.ascontiguousarray(WT[g].reshape(18, 128, 512)) for g in range(B)]

    wf_arr = np.ascontiguousarray(
        w_f.transpose(2, 3, 1, 0).reshape(9, 4, 128, 512).reshape(36, 128, 512))
    ff_arr = np.ascontiguousarray(ffv.reshape(4, 128).T)
    fb_arr = np.ascontiguousarray(fbv.reshape(4, 128).T)
    ident = np.eye(128, dtype=np.float32)
    gmb = np.full((128, 1), float(gamma.reshape(-1)[0]), np.float32)

    # upsample col-mix constants (uniform)
    wc = np.zeros((2, 94), np.float32)
    for jx in range(1, 95):
        xx = jx * 47.0 / 95.0
        x0 = int(np.floor(xx)); wv_ = xx - x0
        u = (jx - 1) // 2
        assert x0 == u, (jx, x0, u)
        wc[0, jx - 1] = 1.0 - wv_
        wc[1, jx - 1] = wv_
    wcol_arr = np.ascontiguousarray(np.broadcast_to(wc[None, :, None, :], (128, 2, 4, 94)))

    in2 = []
    for core in CORES:
        g, i = divmod(core, 4)
        lo = _shard_rows(i)
        # query shard rows lo..lo+16 (clipped, zero pad)
        def shard(a2d, P):  # a2d [P, H, W] -> [P, 16, 48]
            out = np.zeros((P, 16, 48), np.float32)
            y0 = max(0, lo); y1 = min(H, lo + 16)
            out[:, y0 - lo:y1 - lo, :] = a2d[:, y0:y1, :]
            return out
        qsh = shard(q[g].reshape(64, H, W), 64).reshape(64, 768)
        psh = shard(pred[g].reshape(NC, H, W), NC).reshape(NC, 768)
        xsh = shard(x[g].reshape(C, H, W), C).reshape(4, 128, 16, 48)
        # row-mix weights
        wr = np.zeros((26, 3), np.float32)
        for jj in range(26):
            j = 24 * i - 1 + jj
            if 0 <= j < 96:
                yy = j * 47.0 / 95.0
                y0 = int(np.floor(yy)); wv_ = yy - y0
                y0r = y0 - lo
                q0 = jj // 2 + 1
                d = y0r - q0
                assert d in (0, 1), (i, jj, d)
                wr[jj, d] += 1.0 - wv_
                wr[jj, d + 1] += wv_
        wrow_arr = np.ascontiguousarray(np.broadcast_to(wr.reshape(1, 78), (128, 78)))
        in2.append({
            "qs": np.ascontiguousarray(qsh), "kf": np.ascontiguousarray(k[g]),
            "ps": np.ascontiguousarray(psh), "pf": np.ascontiguousarray(pred[g]),
            "wt": wt_arrs[g], "wf": wf_arr, "xs": np.ascontiguousarray(xsh),
            "ident": ident, "gmb": gmb, "wrow": wrow_arr, "wcol": wcol_arr,
            "zp": np.zeros((128, 4, 26, 2), np.float32),
            "ff": ff_arr, "fb": fb_arr,
        })
    r2 = run_bass_kernel_spmd(_L2, in2, CORES, trace=PROFILE)
    out = np.zeros((B, C, 96, 96), np.float32)
    for core in CORES:
        g, i = divmod(core, 4)
        o = r2.results[core]["o_sh"]  # [4,128,24,96]
        out[g, :, 24 * i:24 * i + 24, :] = o.reshape(512, 24, 96)

    kernel.last_exec_ns = (r1.exec_time_ns or 0) + (r2.exec_time_ns or 0) if PROFILE else None
    return out, aux2
